# revision 3
# baseline (speedup 1.0000x reference)
"""Trainium2 Bass kernel for nn_EnhancedTransformerLayer (RoPE attention + MoE).

Sharding: 8 cores; core c -> batch b=c//4, qc=c%4. Four distinct NEFFs (one per
qc), each run on 2 cores (b=0,1). Core qc owns interleaved query blocks
{qc, qc+4, qc+8, qc+12} (4 x 128 tokens) so causal work is balanced, and only
computes K/V up to its last block.

~494us on HW vs the 553us prior baseline. Changes vs that baseline:
- bf16 end-to-end on the QKV path (x, wq/wk/wv, cos/sin, rope, qT, kT):
  halves weight/activation DMA, 2x DVE rope, same 1 cy/row matmul rate.
- V kept in SBUF (V_sb [P, kc, 2*hp+hh, 65] with a ones column feeding the
  fused softmax-denominator row): no DRAM round trip, no per-hp reloads.
- ctx kept in SBUF (ctx_sb [P, dc, QL] bf16): out-proj reads it directly,
  no ctxd DRAM round trip.
- exact-q0 score matmuls (bf16 is 1 cy/row at any N; no 256-pad needed).
- consolidated DMAs (single rearranged dma_start per weight; packed consts;
  packed cos||sin; startup DMAs split per-chunk in rope consumption order and
  spread across sync/gpsimd/scalar queues to cut DGE sequencing).
- expert weights prefetched during attention (ewp pool tiles + DMAs issued
  before phase B so they stream behind it).
- gating softmax denominator via a PE ones-row matmul + one partition
  broadcast (replaces a gpsimd all_reduce).
- MoE x1e pre-scales in per-dc tiles so the first expert matmul only waits
  on its own slice; last expert's matmuls run in column halves so h2+LN2 on
  half 0 overlap the half-1 matmuls; LN2 stores stream per channel.
"""
import sys, os
sys.path.insert(0, '/opt/trn_rl_repo')
import numpy as np
import ml_dtypes

import concourse.bass as bass
from concourse import bacc
import concourse.tile as tile
from concourse import mybir
from concourse import bass_isa

R = mybir.dt.float32r
F = mybir.dt.float32
BF = mybir.dt.bfloat16
P = 128
B, S, E, H, D, NE = 2, 2048, 1024, 16, 64, 8
NC = E // P
QL = 512
EXP_SCALE = 1.0 / (D ** 0.5)
LN_EPS = 1e-5

_cache = {}


def _build(qc):
    nc = bacc.Bacc("TRN2", target_bir_lowering=False, debug=False, num_devices=8,
                   name=f"moe2_qc{qc}", enable_partition_id=False)
    kv_tok = 128 * (qc + 13)
    KCN = kv_tok // P
    # spans of 256 tokens
    spans = []
    s0 = 0
    while s0 < kv_tok:
        sl = min(256, kv_tok - s0)
        spans.append((s0, sl))
        s0 += sl

    def din(name, shape, dt=R):
        return nc.dram_tensor(name, shape, dt, kind="ExternalInput")

    xt = din("xt", [E, S], BF)
    xtq = din("xtq", [E, QL], BF)
    xres = din("xres", [E, QL], BF)
    wq = din("wq", [E, E], BF); wk = din("wk", [E, E], BF); wv = din("wv", [E, E], BF)
    bq = din("bq", [P, NC], F)
    bvr = din("bvr", [1, E], BF)
    wo = din("wo", [E, E], BF)
    gw = din("gw", [E, NE]); gb = din("gb", [NE, 1], F)
    cs2 = din("cs2", [P, 2, S], BF)      # cos||sin packed
    cs2q = din("cs2q", [P, 2, QL], BF)
    trid = din("trid", [P, P], BF)       # tri[k, q] = 1 if q >= k (within a block)
    ew = din("ew", [NE, NC, P, E], BF)
    ebt = din("ebt", [NE, E], BF)
    sels = din("sels", [NE, NE, P], BF)      # sels[k, e, m] = (k == e): row-select stationary
    # packed per-partition consts: bo, ln1w, ln1b, ln2w, ln2b  [P, 5*NC]
    cpack = din("cpack", [P, 5 * NC], F)
    out = nc.dram_tensor("out", [E, QL], R, kind="ExternalOutput")

    AX = mybir.AxisListType.X
    OP = mybir.AluOpType
    AF = mybir.ActivationFunctionType
    import contextlib

    xt_r = xt.rearrange("(c p) s -> p c s", p=P)

    def rope6(dst, src, cos_sb, sin_sb, tmppool, width):
        """dst[:, c] = src[:, c]*cos - src[:, c+4]*sin; dst[:, c+4] = ... + ...
        All reads happen before writes, so dst may alias src (in-place).
        All operands bf16 for 2x DVE throughput."""
        sl = slice(0, width)
        for c in range(4):
            t1 = tmppool.tile([P, width], BF, tag="ropet1")
            t2 = tmppool.tile([P, width], BF, tag="ropet2")
            t3 = tmppool.tile([P, width], BF, tag="ropet3")
            t4 = tmppool.tile([P, width], BF, tag="ropet4")
            nc.vector.tensor_tensor(out=t1[:], in0=src[:, c, sl], in1=cos_sb[:, sl], op=OP.mult)
            nc.vector.tensor_tensor(out=t3[:], in0=src[:, c, sl], in1=sin_sb[:, sl], op=OP.mult)
            nc.vector.tensor_tensor(out=t2[:], in0=src[:, c + 4, sl], in1=sin_sb[:, sl], op=OP.mult)
            nc.vector.tensor_tensor(out=t4[:], in0=src[:, c + 4, sl], in1=cos_sb[:, sl], op=OP.mult)
            nc.vector.tensor_tensor(out=dst[:, c, sl], in0=t1[:], in1=t2[:], op=OP.subtract)
            nc.vector.tensor_tensor(out=dst[:, c + 4, sl], in0=t4[:], in1=t3[:], op=OP.add)

    with tile.TileContext(nc) as tc, \
         nc.allow_low_precision(reason="bf16/float32r path validated against fp32 reference"), \
         contextlib.ExitStack() as es:

        # ===== Phase Q: rope q-chunk + Q projection =====
        # critical-path DMAs first: cs2q, xtq, wq
        attn_res = es.enter_context(tc.tile_pool(name="attn_res", bufs=1))
        qT = attn_res.tile([P, NC, QL], BF, tag="qT")
        kT = attn_res.tile([P, NC, kv_tok], BF, tag="kT")
        V_sb = attn_res.tile([P, KCN, 2 * NC, 65], BF, tag="V_sb")

        consts = es.enter_context(tc.tile_pool(name="consts", bufs=1))
        wkp_cm = tc.tile_pool(name="wkp", bufs=1)
        wkp = wkp_cm.__enter__()
        kvs_cm = tc.tile_pool(name="kvs", bufs=2)
        kvs = kvs_cm.__enter__()

        with tc.tile_pool(name="qph", bufs=1) as qph, \
             tc.tile_pool(name="qtmp", bufs=1) as qtmp, \
             tc.tile_pool(name="qps_p", bufs=4, space="PSUM") as qps_p:
            csq_sb = qph.tile([P, 2, QL], BF, tag="csq")
            nc.gpsimd.dma_start(csq_sb[:], cs2q[:])
            xtq_sb = qph.tile([P, NC, QL], BF, tag="xtq")
            xtq_r = xtq.rearrange("(c p) q -> p c q", p=P)
            for c in [0, 4, 1, 5, 2, 6, 3, 7]:
                nc.gpsimd.dma_start(xtq_sb[:, c, :], xtq_r[:, c, :])
            wq_sb = qph.tile([P, NC, E], BF, tag="wq_sb")
            wq_r = wq.rearrange("(c p) m -> p c m", p=P)
            for c in [0, 4, 1, 5, 2, 6, 3, 7]:
                nc.sync.dma_start(wq_sb[:, c, :], wq_r[:, c, :])

            ones_f = consts.tile([P, 1], F, tag="ones_f")
            nc.vector.memset(ones_f[:], 1.0)
            ones = consts.tile([P, 1], R, tag="ones")
            nc.vector.tensor_copy(out=ones[:], in_=ones_f[:])
            ones1_f = consts.tile([1, P], F, tag="ones1_f")
            nc.vector.memset(ones1_f[:], 1.0)
            ones1b = consts.tile([1, P], BF, tag="ones1b")
            nc.vector.tensor_copy(out=ones1b[:], in_=ones1_f[:])
            eps1 = consts.tile([1, 1], F, tag="eps1")
            nc.vector.memset(eps1[:], LN_EPS)
            ones8f = consts.tile([NE, 1], F, tag="ones8f")
            nc.vector.memset(ones8f[:], 1.0)
            ones8 = consts.tile([NE, 1], R, tag="ones8")
            nc.vector.tensor_copy(out=ones8[:], in_=ones8f[:])
            bq_sb = consts.tile([P, NC], F, tag="bq")
            nc.scalar.dma_start(bq_sb[:], bq[:])
            cpack_sb = consts.tile([P, 5 * NC], F, tag="cpack")
            nc.scalar.dma_start(cpack_sb[:], cpack[:])
            bo_sb = cpack_sb[:, 0 * NC:1 * NC]
            ln_sb = {"ln1w": cpack_sb[:, 1 * NC:2 * NC], "ln1b": cpack_sb[:, 2 * NC:3 * NC],
                     "ln2w": cpack_sb[:, 3 * NC:4 * NC], "ln2b": cpack_sb[:, 4 * NC:5 * NC]}
            tri_sb = consts.tile([P, P], BF, tag="tri")
            nc.scalar.dma_start(tri_sb[:], trid[:])
            gb_sb = consts.tile([NE, 1], F, tag="gb")
            nc.scalar.dma_start(gb_sb[:], gb[:])
            gw_sb = consts.tile([P, NC, NE], R, tag="gw")
            nc.scalar.dma_start(gw_sb[:], gw.rearrange("(c p) g -> p c g", p=P))

            rope6(xtq_sb, xtq_sb, csq_sb[:, 0, :], csq_sb[:, 1, :], qtmp, QL)
            rope_order = [0, 4, 1, 5, 2, 6, 3, 7]
            for oc in range(NC):
                qp = qps_p.tile([P, QL], F, tag="qps")
                for di, dc in enumerate(rope_order):
                    nc.tensor.matmul(
                        qp[:], wq_sb[:, dc, oc * P:(oc + 1) * P], xtq_sb[:, dc, :],
                        start=(di == 0), stop=(di == NC - 1))
                nc.scalar.activation(out=qT[:, oc, :], in_=qp[:],
                                     func=AF.Identity, bias=bq_sb[:, oc:oc + 1])
            wk_sb = wkp.tile([P, NC, E], BF, tag="wk_sb")
            nc.sync.dma_start(wk_sb[:], wk.rearrange("(c p) m -> p c m", p=P))

        # ===== Phase KV: fused rope -> K proj, V proj per 512-token span =====
        with tc.tile_pool(name="kvw", bufs=1) as kvw, \
             tc.tile_pool(name="kvtmp", bufs=2) as kvtmp, \
             tc.tile_pool(name="kps_p", bufs=3, space="PSUM") as kps_p, \
             tc.tile_pool(name="vps_p", bufs=2, space="PSUM") as vps_p:
            wv_sb = kvw.tile([P, NC, E], BF, tag="wv_sb")
            bv_sb = kvw.tile([1, E], BF, tag="bv")
            nc.sync.dma_start(bv_sb[:], bvr[:])
            nc.sync.dma_start(wv_sb[:], wv.rearrange("(c p) m -> p c m", p=P))
            nc.vector.memset(V_sb[:, :, :, 64:65], 1.0)
            for (h0, hl) in spans:
                xsp = kvs.tile([P, NC, 256], BF, tag="xsp")
                nc.sync.dma_start(xsp[:, :, :hl], xt_r[:, :, h0:h0 + hl])
                cssp = kvs.tile([P, 2, 256], BF, tag="cssp", bufs=1)
                nc.sync.dma_start(cssp[:, :, :hl], cs2[:, :, h0:h0 + hl])
                rsp = kvs.tile([P, NC, 256], BF, tag="rsp")
                rope6(rsp, xsp, cssp[:, 0, :], cssp[:, 1, :], kvtmp, hl)
                # K projection (no bias: softmax is invariant to the K bias)
                for oc in range(NC):
                    kp = kps_p.tile([P, 256], F, tag="kps")
                    for dc in range(NC):
                        nc.tensor.matmul(
                            kp[:, :hl], wk_sb[:, dc, oc * P:(oc + 1) * P],
                            rsp[:, dc, :hl], start=(dc == 0), stop=(dc == NC - 1))
                    nc.scalar.copy(out=kT[:, oc, h0:h0 + hl], in_=kp[:, :hl])
                # V projection from the same (un-roped) x span; N=1024 bf16 moving
                for tc_ in range(hl // P):
                    t0 = h0 + tc_ * P
                    kc = t0 // P
                    for dvs in range(2):
                        vp = vps_p.tile([P, 512], F, tag="vps")
                        for dc in range(NC):
                            nc.tensor.matmul(
                                vp[:], xsp[:, dc, tc_ * P:(tc_ + 1) * P],
                                wv_sb[:, dc, dvs * 512:(dvs + 1) * 512],
                                start=(dc == 0), stop=False)
                        nc.tensor.matmul(
                            vp[:], ones1b[:, :], bv_sb[:, dvs * 512:(dvs + 1) * 512],
                            start=False, stop=True)
                        nc.scalar.copy(out=V_sb[:, kc, 8 * dvs:8 * dvs + 8, 0:64],
                                       in_=vp[:])
        kvs_cm.__exit__(None, None, None)
        wkp_cm.__exit__(None, None, None)

        # ctx output lives past attention into phase C
        ctxsb_pool = es.enter_context(tc.tile_pool(name="ctxsb", bufs=1))
        ctx_sb = ctxsb_pool.tile([P, NC, QL], BF, tag="ctx_sb")

        # prefetch pools for phase C/D (DMAs issued before attention so they
        # stream during it)
        cres = es.enter_context(tc.tile_pool(name="cres", bufs=1))
        x1 = cres.tile([P, NC, QL], R, tag="x1")   # holds h1, then LN1 output
        wbc = cres.tile([P, NE, QL], BF, tag="wbc")
        wfin = cres.tile([NE, QL], BF, tag="wfin")
        ebt_sb = cres.tile([NE, E], BF, tag="ebt")
        nc.sync.dma_start(ebt_sb[:], ebt[:])
        sels_sb = cres.tile([NE, NE, P], BF, tag="sels")
        nc.sync.dma_start(sels_sb[:], sels[:])
        wo_sb = cres.tile([P, NC, E], BF, tag="wo_sb")
        nc.sync.dma_start(wo_sb[:], wo.rearrange("(c p) m -> p c m", p=P))
        xres_sb = cres.tile([P, NC, QL], BF, tag="xres")
        nc.sync.dma_start(xres_sb[:], xres.rearrange("(c p) q -> p c q", p=P))
        ewp = es.enter_context(tc.tile_pool(name="ewp", bufs=2))
        ew_tiles = []
        for e in range(NE):
            t = ewp.tile([P, NC, E], BF, tag="ew_sl", name=f"ew{e}")
            nc.sync.dma_start(t[:], ew[e].rearrange("c p d -> p c d"))
            ew_tiles.append(t)

        # ===== Phase B: causal attention, hh-paired exp, fused denominator =====
        with tc.tile_pool(name="st_p", bufs=3) as st_p, \
             tc.tile_pool(name="rd_p", bufs=2) as rd_p, \
             tc.tile_pool(name="rb_p", bufs=2) as rb_p, \
             tc.tile_pool(name="scps", bufs=2, space="PSUM") as scps, \
             tc.tile_pool(name="ctxps", bufs=2, space="PSUM") as ctxps:
            for hp in range(NC):
                ctxp2 = [ctxps.tile([65, QL], F, tag=f"ctx{hh}", name=f"ctxp_{hp}_{hh}")
                         for hh in range(2)]
                prev = None
                for kc in range(KCN):
                    j0 = max(0, -(-(kc - qc) // 4))
                    q0 = j0 * P
                    scp = scps.tile([P, 2, QL], F, tag="scp")
                    for hh in range(2):
                        nc.tensor.matmul(
                            scp[:, hh, q0:], kT[hh * 64:(hh + 1) * 64, hp, kc * P:(kc + 1) * P],
                            qT[hh * 64:(hh + 1) * 64, hp, q0:], start=True, stop=True)
                    if prev is not None:
                        pkc, pq0, pst = prev
                        for hh in range(2):
                            nc.tensor.matmul(ctxp2[hh][:, pq0:], V_sb[:, pkc, 2 * hp + hh, :],
                                             pst[:, hh, pq0:],
                                             start=(pkc == 0), stop=False)
                    st = st_p.tile([P, 2, QL], BF, tag="st")
                    nc.scalar.activation(out=st[:, :, q0:], in_=scp[:, :, q0:],
                                         func=AF.Exp, scale=EXP_SCALE)
                    if kc >= qc and (kc - qc) % 4 == 0:
                        j = (kc - qc) // 4
                        for hh in range(2):
                            nc.vector.tensor_tensor(
                                out=st[:, hh, j * P:(j + 1) * P],
                                in0=st[:, hh, j * P:(j + 1) * P],
                                in1=tri_sb[:], op=OP.mult)
                    prev = (kc, q0, st)
                pkc, pq0, pst = prev
                for hh in range(2):
                    nc.tensor.matmul(ctxp2[hh][:, pq0:], V_sb[:, pkc, 2 * hp + hh, :],
                                     pst[:, hh, pq0:], start=(pkc == 0), stop=True)
                for hh in range(2):
                    dsb = rd_p.tile([1, QL], F, tag="dsb")
                    nc.vector.tensor_copy(out=dsb[:], in_=ctxp2[hh][64:65, :])
                    rden = rd_p.tile([1, QL], F, tag="rden")
                    nc.vector.reciprocal_approx_fast(out=rden[:], in_=dsb[:])
                    rb = rb_p.tile([64, QL], F, tag="rb")
                    nc.gpsimd.partition_broadcast(rb[:], rden[:])
                    nc.vector.tensor_tensor(
                        out=ctx_sb[hh * 64:(hh + 1) * 64, hp, :],
                        in0=ctxp2[hh][0:64, :], in1=rb[:], op=OP.mult)

        # ===== LN helper =====
        def layernorm(src, dst, wtile, btile, tmp, ps_row, post_c=None, cols=slice(None)):
            sp_ = ps_row.tile([1, QL], F, tag="lnrow")
            for c in range(NC):
                nc.tensor.matmul(sp_[:, cols], ones[:], src[:, c, cols],
                                 start=(c == 0), stop=(c == NC - 1))
            s2p = ps_row.tile([1, QL], F, tag="lnrow2")
            for c in range(NC):
                sq = tmp.tile([P, QL], R, tag="lnsq", bufs=2)
                nc.scalar.activation(out=sq[:, cols], in_=src[:, c, cols], func=AF.Square)
                nc.tensor.matmul(s2p[:, cols], ones[:], sq[:, cols],
                                 start=(c == 0), stop=(c == NC - 1))
            mean = tmp.tile([1, QL], F, tag="lnmean")
            nc.scalar.mul(out=mean[:, cols], in_=sp_[:, cols], mul=1.0 / E)
            msq = tmp.tile([1, QL], R, tag="lnmsq")
            nc.vector.tensor_tensor(out=msq[:, cols], in0=mean[:, cols], in1=mean[:, cols], op=OP.mult)
            var = tmp.tile([1, QL], R, tag="lnvar")
            nc.vector.scalar_tensor_tensor(out=var[:, cols], in0=s2p[:, cols], scalar=1.0 / E,
                                           in1=msq[:, cols], op0=OP.mult, op1=OP.subtract)
            std = tmp.tile([1, QL], F, tag="lnstd")
            nc.scalar.activation(out=std[:, cols], in_=var[:, cols], func=AF.Sqrt, bias=eps1[:])
            rstd = tmp.tile([1, QL], F, tag="lnrstd")
            nc.vector.reciprocal_approx_fast(out=rstd[:, cols], in_=std[:, cols])
            mb = tmp.tile([P, QL], F, tag="lnmb")
            nc.gpsimd.partition_broadcast(mb[:, cols], mean[:, cols])
            rbb = tmp.tile([P, QL], F, tag="lnrb")
            nc.gpsimd.partition_broadcast(rbb[:, cols], rstd[:, cols])
            for c in range(NC):
                t = tmp.tile([P, QL], R, tag="lnt", bufs=2)
                nc.vector.tensor_tensor(out=t[:, cols], in0=src[:, c, cols], in1=mb[:, cols], op=OP.subtract)
                nc.vector.tensor_tensor(out=t[:, cols], in0=t[:, cols], in1=rbb[:, cols], op=OP.mult)
                nc.vector.tensor_scalar(out=dst[:, c, cols], in0=t[:, cols],
                                        scalar1=wtile[:, c:c + 1], scalar2=btile[:, c:c + 1],
                                        op0=OP.mult, op1=OP.add)
                if post_c is not None:
                    post_c(c)

        # ===== Phase C: out-proj + LN1 (in place: h1 -> x1) + gating =====
        with tc.tile_pool(name="ct", bufs=1) as ct, \
             tc.tile_pool(name="cps8", bufs=1, space="PSUM") as cps8:
            aps = [cps8.tile([P, QL], F, tag=f"ap{oc}", name=f"ap{oc}")
                   for oc in range(NC)]
            for dc in range(NC):
                for oc in range(NC):
                    nc.tensor.matmul(aps[oc][:], wo_sb[:, dc, oc * P:(oc + 1) * P],
                                     ctx_sb[:, dc, :], start=(dc == 0), stop=(dc == NC - 1))
            for oc in range(NC):
                nc.vector.scalar_tensor_tensor(
                    out=x1[:, oc, :], in0=aps[oc][:], scalar=bo_sb[:, oc:oc + 1],
                    in1=xres_sb[:, oc, :], op0=OP.add, op1=OP.add)
        with tc.tile_pool(name="ct2", bufs=1) as ct, \
             tc.tile_pool(name="cps", bufs=2, space="PSUM") as cps, \
             tc.tile_pool(name="crow", bufs=2, space="PSUM") as crow:
            layernorm(x1, x1, ln_sb["ln1w"], ln_sb["ln1b"], ct, crow)

            # gating in [NE, QL] layout
            gp = crow.tile([NE, QL], F, tag="gps")
            for c in range(NC):
                nc.tensor.matmul(gp[:], gw_sb[:, c, :], x1[:, c, :],
                                 start=(c == 0), stop=(c == NC - 1))
            gexp = ct.tile([NE, QL], R, tag="gexp")
            nc.scalar.activation(out=gexp[:], in_=gp[:], func=AF.Exp, bias=gb_sb[:])
            denp = crow.tile([1, QL], F, tag="gps", name="denp")
            nc.tensor.matmul(denp[:], ones8[:], gexp[:], start=True, stop=True)
            den1 = ct.tile([1, QL], F, tag="den1")
            nc.vector.tensor_copy(out=den1[:], in_=denp[:])
            nc.vector.reciprocal_approx_fast(out=den1[:], in_=den1[:])
            rgb = ct.tile([NE, QL], F, tag="rgb")
            nc.gpsimd.partition_broadcast(rgb[:], den1[:])

            m1b = ct.tile([NE, QL], R, tag="gtmp", bufs=3)
            nc.gpsimd.partition_all_reduce(m1b[:], gexp[:], channels=NE,
                                           reduce_op=bass_isa.ReduceOp.max)
            msel = ct.tile([NE, QL], R, tag="msel")
            nc.vector.tensor_tensor(out=msel[:], in0=gexp[:], in1=m1b[:], op=OP.is_equal)
            inv = ct.tile([NE, QL], R, tag="gtmp", bufs=3)
            nc.vector.tensor_scalar(out=inv[:], in0=msel[:], scalar1=-1.0, scalar2=1.0,
                                    op0=OP.mult, op1=OP.add)
            g2 = ct.tile([NE, QL], R, tag="gtmp", bufs=3)
            nc.vector.tensor_tensor(out=g2[:], in0=gexp[:], in1=inv[:], op=OP.mult)
            m2b = ct.tile([NE, QL], R, tag="gtmp", bufs=3)
            nc.gpsimd.partition_all_reduce(m2b[:], g2[:], channels=NE,
                                           reduce_op=bass_isa.ReduceOp.max)
            msel2 = ct.tile([NE, QL], R, tag="gtmp", bufs=3)
            nc.vector.tensor_tensor(out=msel2[:], in0=g2[:], in1=m2b[:], op=OP.is_equal)
            nc.vector.tensor_tensor(out=msel[:], in0=msel[:], in1=msel2[:], op=OP.add)
            wsel = ct.tile([NE, QL], R, tag="wsel")
            nc.vector.tensor_tensor(out=wsel[:], in0=gexp[:], in1=msel[:], op=OP.mult)
            nc.vector.tensor_tensor(out=wfin[:], in0=wsel[:], in1=rgb[:], op=OP.mult)
            for e in range(NE):
                wbp = cps.tile([P, QL], F, tag="cbig")
                nc.tensor.matmul(wbp[:], sels_sb[:, e, :], wfin[:], start=True, stop=True)
                nc.vector.tensor_copy(out=wbc[:, e, :], in_=wbp[:])

        # ===== Phase D: MoE, experts accumulate in PSUM =====
        h2 = x1  # x1 is dead after the residual add; reuse its SBUF
        with tc.tile_pool(name="x1e_p", bufs=2) as x1e_p, \
             tc.tile_pool(name="dps", bufs=1, space="PSUM") as dps:
            yps = [dps.tile([P, QL], F, tag=f"yp{oc}", name=f"yp{oc}") for oc in range(NC)]
            for oc in range(NC):
                nc.tensor.matmul(yps[oc][:], ebt_sb[:, oc * P:(oc + 1) * P], wfin[:],
                                 start=True, stop=False)
            for e in range(NE):
                ew_sl = ew_tiles[e]
                x1e = [x1e_p.tile([P, QL], BF, tag=f"x1e{dc}", bufs=2, name=f"x1e_{e}_{dc}") for dc in range(NC)]
                for dc in range(NC):
                    nc.vector.tensor_tensor(out=x1e[dc][:], in0=x1[:, dc, :],
                                            in1=wbc[:, e, :], op=OP.mult)
                if e < NE - 1:
                    for dc in range(NC):
                        for oc in range(NC):
                            nc.tensor.matmul(
                                yps[oc][:], ew_sl[:, dc, oc * P:(oc + 1) * P], x1e[dc][:],
                                start=False, stop=False)
                else:
                    # last expert: finish column-half 0 first so h2+LN2 on it
                    # overlap with the half-1 matmuls
                    for h0, h1 in ((0, 256), (256, QL)):
                        for dc in range(NC):
                            for oc in range(NC):
                                nc.tensor.matmul(
                                    yps[oc][:, h0:h1], ew_sl[:, dc, oc * P:(oc + 1) * P],
                                    x1e[dc][:, h0:h1],
                                    start=False, stop=(dc == NC - 1))
            for h0, h1 in ((0, 256), (256, QL)):
                for oc in range(NC):
                    nc.vector.tensor_tensor(out=h2[:, oc, h0:h1], in0=yps[oc][:, h0:h1],
                                            in1=x1[:, oc, h0:h1], op=OP.add)

        # ===== Phase E: LN2 + store =====
        with tc.tile_pool(name="et", bufs=1) as et, \
             tc.tile_pool(name="erow", bufs=2, space="PSUM") as erow:
            out_r = out.rearrange("(c p) q -> p c q", p=P)
            for h0, h1 in ((0, 256), (256, QL)):
                layernorm(h2, h2, ln_sb["ln2w"], ln_sb["ln2b"], et, erow,
                          post_c=(lambda h0=h0, h1=h1: (lambda c: nc.sync.dma_start(
                              out_r[:, c, h0:h1], h2[:, c, h0:h1])))(),
                          cols=slice(h0, h1))

    nc.compile()
    return nc


def _prep_inputs(inputs):
    x = np.asarray(inputs['x'], dtype=np.float32)
    ipw = np.asarray(inputs['in_proj_w'], dtype=np.float32)
    ipb = np.asarray(inputs['in_proj_b'], dtype=np.float32)
    opw = np.asarray(inputs['out_proj_w'], dtype=np.float32)
    opb = np.asarray(inputs['out_proj_b'], dtype=np.float32)
    gww = np.asarray(inputs['gate_w'], dtype=np.float32)
    gbb = np.asarray(inputs['gate_b'], dtype=np.float32)
    eww = np.asarray(inputs['expert_w'], dtype=np.float32)
    ebb = np.asarray(inputs['expert_b'], dtype=np.float32)

    perm = np.empty(E, dtype=np.int64)
    idx = 0
    for h in range(H):
        for i in range(D // 2):
            perm[idx] = 64 * h + 2 * i; idx += 1
    for h in range(H):
        for i in range(D // 2):
            perm[idx] = 64 * h + 2 * i + 1; idx += 1

    Wq, Wk, Wv = ipw[0:E], ipw[E:2 * E], ipw[2 * E:3 * E]
    bq_, bk_, bv_ = ipb[0:E], ipb[E:2 * E], ipb[2 * E:3 * E]
    BFD = ml_dtypes.bfloat16
    common = {
        "wq": np.ascontiguousarray(Wq[:, perm].T).astype(BFD),
        "wk": np.ascontiguousarray(Wk[:, perm].T).astype(BFD),
        "wv": np.ascontiguousarray(Wv[:, perm].T).astype(BFD),
        "bq": np.ascontiguousarray(bq_.reshape(NC, P).T),
        "bvr": bv_.reshape(1, E).astype(BFD),
        "wo": np.ascontiguousarray(opw.T).astype(BFD),
        "gw": np.ascontiguousarray(gww.T),
        "gb": gbb.reshape(NE, 1).copy(),
        "ew": np.ascontiguousarray(
            eww.transpose(0, 2, 1).reshape(NE, NC, P, E)).astype(BFD),
        "ebt": np.ascontiguousarray(ebb).astype(BFD),
        "trid": np.ascontiguousarray(
            (np.arange(P)[None, :] >= np.arange(P)[:, None])).astype(BFD),
        "sels": np.ascontiguousarray(
            np.repeat(np.eye(NE, dtype=np.float32)[:, :, None], P, axis=2)).astype(BFD),
        "cpack": np.ascontiguousarray(np.concatenate([
            opb.reshape(NC, P).T,
            np.asarray(inputs['ln1_w'], np.float32).reshape(NC, P).T,
            np.asarray(inputs['ln1_b'], np.float32).reshape(NC, P).T,
            np.asarray(inputs['ln2_w'], np.float32).reshape(NC, P).T,
            np.asarray(inputs['ln2_b'], np.float32).reshape(NC, P).T,
        ], axis=1)),
    }
    inv_freq = 1.0 / (10000.0 ** (np.arange(0, D, 2, dtype=np.float64) / D))
    freqs = np.arange(S, dtype=np.float64)[:, None] * inv_freq[None, :]
    cos_t = np.cos(freqs).T.astype(np.float32)
    sin_t = np.sin(freqs).T.astype(np.float32)
    cos2 = np.ascontiguousarray(np.tile(cos_t, (4, 1)))
    sin2 = np.ascontiguousarray(np.tile(sin_t, (4, 1)))
    cs2 = np.stack([cos2, sin2], axis=1)  # [P, 2, S]
    common["cs2"] = np.ascontiguousarray(cs2).astype(BFD)

    in_maps = []
    for c in range(8):
        b, qc = c // 4, c % 4
        blocks = [qc + 4 * i for i in range(4)]
        cols = np.concatenate([np.arange(blk * P, (blk + 1) * P) for blk in blocks])
        xtb = np.ascontiguousarray(x[b].T)
        xtp = np.ascontiguousarray(xtb[perm])
        m = dict(common)
        m["xt"] = xtp.astype(BFD)
        m["xtq"] = np.ascontiguousarray(xtp[:, cols]).astype(BFD)
        m["xres"] = np.ascontiguousarray(xtb[:, cols]).astype(BFD)
        m["cs2q"] = np.ascontiguousarray(cs2[:, :, cols]).astype(BFD)
        in_maps.append(m)
    return in_maps


def _run_multi(ncs, in_maps):
    """Run the 4 NEFFs concurrently: graph qc on devices {qc, qc+4} (b=0,1)."""
    import jax
    from jax.sharding import Mesh, PartitionSpec
    from jax.experimental.shard_map import shard_map
    from concourse import bass2jax
    from concourse import mybir as _mb

    bass2jax.install_neuronx_cc_hook()
    devices = jax.devices()

    if "jits" not in _cache:
        _cache["jits"] = {}
    handles = []
    for qc in range(4):
        nc = ncs[qc]
        if qc not in _cache["jits"]:
            in_names, out_names, out_avals, zero_outs = [], [], [], []
            for alloc in nc.m.functions[0].allocations:
                if not isinstance(alloc, _mb.MemoryLocationSet):
                    continue
                name = alloc.memorylocations[0].name
                if alloc.kind == "ExternalInput":
                    in_names.append(name)
                elif alloc.kind == "ExternalOutput":
                    out_names.append(name)
                    shape = tuple(alloc.tensor_shape)
                    dtype = _mb.dt.np(alloc.dtype)
                    out_avals.append(jax.core.ShapedArray(shape, dtype))
                    zero_outs.append(np.zeros(shape, dtype))
            n_params = len(in_names)
            all_names = in_names + out_names
            donate = tuple(range(n_params, n_params + len(out_names)))

            def _body(*args, _nc=nc, _avals=tuple(out_avals), _all=tuple(all_names),
                      _outs=tuple(out_names)):
                outs = bass2jax._bass_exec_p.bind(
                    *args, out_avals=_avals, in_names=_all, out_names=_outs,
                    lowering_input_output_aliases=(),
                    sim_require_finite=True, sim_require_nnan=True, nc=_nc)
                return tuple(outs)

            devs = [devices[qc], devices[qc + 4]]
            mesh = Mesh(np.asarray(devs), ("core",))
            nio = n_params + len(zero_outs)
            sharded = jax.jit(
                shard_map(_body, mesh=mesh,
                          in_specs=(PartitionSpec("core"),) * nio,
                          out_specs=(PartitionSpec("core"),) * len(out_names),
                          check_rep=False),
                donate_argnums=donate, keep_unused=True)
            _cache["jits"][qc] = (sharded, in_names, out_names, zero_outs)
        sharded, in_names, out_names, zero_outs = _cache["jits"][qc]
        per_core = [[np.asarray(in_maps[b * 4 + qc][n]) for n in in_names] for b in range(2)]
        concat_in = [np.concatenate([per_core[b][i] for b in range(2)], axis=0)
                     for i in range(len(in_names))]
        concat_zero = [np.concatenate([z, z], axis=0) for z in zero_outs]
        handles.append((sharded, concat_in, concat_zero, out_names))

    outs = []
    for sharded, concat_in, concat_zero, out_names in handles:
        outs.append((sharded(*concat_in, *concat_zero), out_names))
    results = [None] * 8
    for qc, (arrs, out_names) in enumerate(outs):
        arrs = [np.asarray(a) for a in arrs]
        for b in range(2):
            rm = {}
            for i, n in enumerate(out_names):
                full = arrs[i]
                half = full.shape[0] // 2
                rm[n] = full[b * half:(b + 1) * half]
            results[b * 4 + qc] = rm
    return results


def _ensure_ntff_hook():
    import types
    try:
        from antenv.axon_hooks import get_axon_ntff_profile_hook  # noqa
        return True
    except ImportError:
        pass
    try:
        import antenv
        sys.path.insert(0, '/root/.axon_site')
        from trn_agent_boot.trn_boot import _ntff_profile_via_ctypes
        hook = _ntff_profile_via_ctypes('/opt/axon/libaxon_pjrt.so')
        if hook is None:
            return False
        mod = types.ModuleType('antenv.axon_hooks')
        _state = {'hook': hook}
        mod.set_axon_ntff_profile_hook = lambda h: _state.__setitem__('hook', h)
        mod.get_axon_ntff_profile_hook = lambda: _state['hook']
        sys.modules['antenv.axon_hooks'] = mod
        antenv.axon_hooks = mod
        return True
    except Exception as e:
        print(f"ntff hook setup failed: {e}")
        return False


def kernel(**inputs):
    if "ncs" not in _cache:
        _cache["ncs"] = [_build(qc) for qc in range(4)]
    ncs = _cache["ncs"]
    in_maps = _prep_inputs(inputs)

    trace = bool(int(os.environ.get("KERNEL_TRACE", "0")))
    if trace and _ensure_ntff_hook():
        import tempfile
        from antenv.axon_hooks import get_axon_ntff_profile_hook
        hook = get_axon_ntff_profile_hook()
        tmpdir = tempfile.mkdtemp()
        _run_multi(ncs, in_maps)  # warm-up/compile outside the profile window
        with hook(tmpdir, list(range(8))):
            results = _run_multi(ncs, in_maps)
        _cache["ntff_dir"] = tmpdir
        print(f"ntff dir: {tmpdir}")
    else:
        results = _run_multi(ncs, in_maps)
    _cache["last_results"] = results

    out = np.empty((B, S, E), dtype=np.float32)
    for c in range(8):
        b, qc = c // 4, c % 4
        o = results[c]["out"]  # [E, QL]
        for i in range(4):
            blk = qc + 4 * i
            out[b, blk * P:(blk + 1) * P, :] = o[:, i * P:(i + 1) * P].T
    return out


# revision 5
# speedup vs baseline: 1.0315x; 1.0315x over previous
"""Trainium2 Bass kernel for nn_EnhancedTransformerLayer (RoPE attention + MoE).

Sharding: 8 cores; core c -> batch b=c//4, qc=c%4. Four distinct NEFFs (one per
qc), each run on 2 cores (b=0,1). Core qc owns interleaved query blocks
{qc, qc+4, qc+8, qc+12} (4 x 128 tokens) so causal work is balanced, and only
computes K/V up to its last block.

~494us on HW vs the 553us prior baseline. Changes vs that baseline:
- bf16 end-to-end on the QKV path (x, wq/wk/wv, cos/sin, rope, qT, kT):
  halves weight/activation DMA, 2x DVE rope, same 1 cy/row matmul rate.
- V kept in SBUF (V_sb [P, kc, 2*hp+hh, 65] with a ones column feeding the
  fused softmax-denominator row): no DRAM round trip, no per-hp reloads.
- ctx kept in SBUF (ctx_sb [P, dc, QL] bf16): out-proj reads it directly,
  no ctxd DRAM round trip.
- exact-q0 score matmuls (bf16 is 1 cy/row at any N; no 256-pad needed).
- consolidated DMAs (single rearranged dma_start per weight; packed consts;
  packed cos||sin; startup DMAs split per-chunk in rope consumption order and
  spread across sync/gpsimd/scalar queues to cut DGE sequencing).
- expert weights prefetched during attention (ewp pool tiles + DMAs issued
  before phase B so they stream behind it).
- gating softmax denominator via a PE ones-row matmul + one partition
  broadcast (replaces a gpsimd all_reduce).
- MoE x1e pre-scales in per-dc tiles so the first expert matmul only waits
  on its own slice; last expert's matmuls run in column halves so h2+LN2 on
  half 0 overlap the half-1 matmuls; LN2 stores stream per channel.
"""
import sys, os
sys.path.insert(0, '/opt/trn_rl_repo')
import numpy as np
import ml_dtypes

import concourse.bass as bass
from concourse import bacc
import concourse.tile as tile
from concourse import mybir
from concourse import bass_isa

R = mybir.dt.float32r
F = mybir.dt.float32
BF = mybir.dt.bfloat16
P = 128
B, S, E, H, D, NE = 2, 2048, 1024, 16, 64, 8
NC = E // P
QL = 512
EXP_SCALE = 1.0 / (D ** 0.5)
LN_EPS = 1e-5

_cache = {}


def _build(qc):
    nc = bacc.Bacc("TRN2", target_bir_lowering=False, debug=False, num_devices=8,
                   name=f"moe2_qc{qc}", enable_partition_id=False)
    kv_tok = 128 * (qc + 13)
    KCN = kv_tok // P
    # spans of 256 tokens
    spans = []
    s0 = 0
    while s0 < kv_tok:
        sl = min(256, kv_tok - s0)
        spans.append((s0, sl))
        s0 += sl

    def din(name, shape, dt=R):
        return nc.dram_tensor(name, shape, dt, kind="ExternalInput")

    xt = din("xt", [E, S], BF)
    xtq = din("xtq", [E, QL], BF)
    xres = din("xres", [E, QL], BF)
    wq = din("wq", [E, E], BF); wk = din("wk", [E, E], BF); wv = din("wv", [E, E], BF)
    bq = din("bq", [P, NC], F)
    wo = din("wo", [E, E], BF)
    gw = din("gw", [E, NE]); gb = din("gb", [NE, 1], F)
    cs2 = din("cs2", [P, 2, S], BF)      # cos||sin packed
    cs2q = din("cs2q", [P, 2, QL], BF)
    trid = din("trid", [P, P], BF)       # tri[k, q] = 1 if q >= k (within a block)
    ew = din("ew", [NE, NC, P, E], BF)
    ebt = din("ebt", [NE, E], BF)
    sels = din("sels", [NE, NE, P], BF)      # sels[k, e, m] = (k == e): row-select stationary
    # packed per-partition consts: bo, ln1w, ln1b, ln2w, ln2b  [P, 5*NC]
    cpack = din("cpack", [P, 6 * NC], F)
    out = nc.dram_tensor("out", [E, QL], R, kind="ExternalOutput")

    AX = mybir.AxisListType.X
    OP = mybir.AluOpType
    AF = mybir.ActivationFunctionType
    import contextlib

    xt_r = xt.rearrange("(c p) s -> p c s", p=P)

    def rope6(dst, src, cos_sb, sin_sb, tmppool, width):
        """dst[:, c] = src[:, c]*cos - src[:, c+4]*sin; dst[:, c+4] = ... + ...
        All reads happen before writes, so dst may alias src (in-place).
        All operands bf16 for 2x DVE throughput."""
        sl = slice(0, width)
        for c in range(4):
            t1 = tmppool.tile([P, width], BF, tag="ropet1")
            t2 = tmppool.tile([P, width], BF, tag="ropet2")
            t3 = tmppool.tile([P, width], BF, tag="ropet3")
            t4 = tmppool.tile([P, width], BF, tag="ropet4")
            nc.vector.tensor_tensor(out=t1[:], in0=src[:, c, sl], in1=cos_sb[:, sl], op=OP.mult)
            nc.vector.tensor_tensor(out=t3[:], in0=src[:, c, sl], in1=sin_sb[:, sl], op=OP.mult)
            nc.vector.tensor_tensor(out=t2[:], in0=src[:, c + 4, sl], in1=sin_sb[:, sl], op=OP.mult)
            nc.vector.tensor_tensor(out=t4[:], in0=src[:, c + 4, sl], in1=cos_sb[:, sl], op=OP.mult)
            nc.vector.tensor_tensor(out=dst[:, c, sl], in0=t1[:], in1=t2[:], op=OP.subtract)
            nc.vector.tensor_tensor(out=dst[:, c + 4, sl], in0=t4[:], in1=t3[:], op=OP.add)

    with tile.TileContext(nc) as tc, \
         nc.allow_low_precision(reason="bf16/float32r path validated against fp32 reference"), \
         contextlib.ExitStack() as es:

        # ===== Phase Q: rope q-chunk + Q projection =====
        # critical-path DMAs first: cs2q, xtq, wq
        attn_res = es.enter_context(tc.tile_pool(name="attn_res", bufs=1))
        qT = attn_res.tile([P, NC, QL], BF, tag="qT")
        kT = attn_res.tile([P, NC, kv_tok], BF, tag="kT")
        V_sb = attn_res.tile([P, KCN, 2 * NC, 65], BF, tag="V_sb")

        consts = es.enter_context(tc.tile_pool(name="consts", bufs=1))
        wkp_cm = tc.tile_pool(name="wkp", bufs=1)
        wkp = wkp_cm.__enter__()
        kvs_cm = tc.tile_pool(name="kvs", bufs=2)
        kvs = kvs_cm.__enter__()

        with tc.tile_pool(name="qph", bufs=1) as qph, \
             tc.tile_pool(name="qtmp", bufs=1) as qtmp, \
             tc.tile_pool(name="qps_p", bufs=4, space="PSUM") as qps_p:
            csq_sb = qph.tile([P, 2, QL], BF, tag="csq")
            nc.gpsimd.dma_start(csq_sb[:], cs2q[:])
            xtq_sb = qph.tile([P, NC, QL], BF, tag="xtq")
            xtq_r = xtq.rearrange("(c p) q -> p c q", p=P)
            for c in [0, 4, 1, 5, 2, 6, 3, 7]:
                nc.gpsimd.dma_start(xtq_sb[:, c, :], xtq_r[:, c, :])
            wq_sb = qph.tile([P, NC, E], BF, tag="wq_sb")
            wq_r = wq.rearrange("(c p) m -> p c m", p=P)
            for c in [0, 4, 1, 5, 2, 6, 3, 7]:
                nc.sync.dma_start(wq_sb[:, c, :], wq_r[:, c, :])

            ones_f = consts.tile([P, 1], F, tag="ones_f")
            nc.vector.memset(ones_f[:], 1.0)
            ones = consts.tile([P, 1], R, tag="ones")
            nc.vector.tensor_copy(out=ones[:], in_=ones_f[:])
            eps1 = consts.tile([1, 1], F, tag="eps1")
            nc.vector.memset(eps1[:], LN_EPS)
            ones8f = consts.tile([NE, 1], F, tag="ones8f")
            nc.vector.memset(ones8f[:], 1.0)
            ones8 = consts.tile([NE, 1], R, tag="ones8")
            nc.vector.tensor_copy(out=ones8[:], in_=ones8f[:])
            bq_sb = consts.tile([P, NC], F, tag="bq")
            nc.scalar.dma_start(bq_sb[:], bq[:])
            cpack_sb = consts.tile([P, 6 * NC], F, tag="cpack")
            nc.scalar.dma_start(cpack_sb[:], cpack[:])
            bo_sb = cpack_sb[:, 0 * NC:1 * NC]
            ln_sb = {"ln1w": cpack_sb[:, 1 * NC:2 * NC], "ln1b": cpack_sb[:, 2 * NC:3 * NC],
                     "ln2w": cpack_sb[:, 3 * NC:4 * NC], "ln2b": cpack_sb[:, 4 * NC:5 * NC]}
            bvp_sb = cpack_sb[:, 5 * NC:6 * NC]
            tri_sb = consts.tile([P, P], BF, tag="tri")
            nc.scalar.dma_start(tri_sb[:], trid[:])
            gb_sb = consts.tile([NE, 1], F, tag="gb")
            nc.scalar.dma_start(gb_sb[:], gb[:])
            gw_sb = consts.tile([P, NC, NE], R, tag="gw")
            nc.scalar.dma_start(gw_sb[:], gw.rearrange("(c p) g -> p c g", p=P))

            rope6(xtq_sb, xtq_sb, csq_sb[:, 0, :], csq_sb[:, 1, :], qtmp, QL)
            rope_order = [0, 4, 1, 5, 2, 6, 3, 7]
            for oc in range(NC):
                qp = qps_p.tile([P, QL], F, tag="qps")
                for di, dc in enumerate(rope_order):
                    nc.tensor.matmul(
                        qp[:], wq_sb[:, dc, oc * P:(oc + 1) * P], xtq_sb[:, dc, :],
                        start=(di == 0), stop=(di == NC - 1))
                nc.scalar.activation(out=qT[:, oc, :], in_=qp[:],
                                     func=AF.Identity, bias=bq_sb[:, oc:oc + 1])
            wk_sb = wkp.tile([P, NC, E], BF, tag="wk_sb")
            nc.sync.dma_start(wk_sb[:], wk.rearrange("(c p) m -> p c m", p=P))

        # ===== Phase KV: fused rope -> K proj, V proj per 512-token span =====
        with tc.tile_pool(name="kvw", bufs=1) as kvw, \
             tc.tile_pool(name="kvtmp", bufs=2) as kvtmp, \
             tc.tile_pool(name="kps_p", bufs=3, space="PSUM") as kps_p, \
             tc.tile_pool(name="vps_p", bufs=2, space="PSUM") as vps_p:
            wv_sb = kvw.tile([P, NC, E], BF, tag="wv_sb")
            nc.sync.dma_start(wv_sb[:], wv.rearrange("(c p) m -> p c m", p=P))
            nc.vector.memset(V_sb[:, :, :, 64:65], 1.0)
            for (h0, hl) in spans:
                xsp = kvs.tile([P, NC, 256], BF, tag="xsp")
                nc.sync.dma_start(xsp[:, :, :hl], xt_r[:, :, h0:h0 + hl])
                cssp = kvs.tile([P, 2, 256], BF, tag="cssp", bufs=1)
                nc.sync.dma_start(cssp[:, :, :hl], cs2[:, :, h0:h0 + hl])
                rsp = kvs.tile([P, NC, 256], BF, tag="rsp")
                rope6(rsp, xsp, cssp[:, 0, :], cssp[:, 1, :], kvtmp, hl)
                # K projection (no bias: softmax is invariant to the K bias)
                for oc in range(NC):
                    kp = kps_p.tile([P, 256], F, tag="kps")
                    for dc in range(NC):
                        nc.tensor.matmul(
                            kp[:, :hl], wk_sb[:, dc, oc * P:(oc + 1) * P],
                            rsp[:, dc, :hl], start=(dc == 0), stop=(dc == NC - 1))
                    nc.scalar.copy(out=kT[:, oc, h0:h0 + hl], in_=kp[:, :hl])
                # V projection from the same (un-roped) x span; N=1024 bf16 moving
                for tc_ in range(hl // P):
                    t0 = h0 + tc_ * P
                    kc = t0 // P
                    for dvs in range(2):
                        vp = vps_p.tile([P, 512], F, tag="vps")
                        for dc in range(NC):
                            nc.tensor.matmul(
                                vp[:], xsp[:, dc, tc_ * P:(tc_ + 1) * P],
                                wv_sb[:, dc, dvs * 512:(dvs + 1) * 512],
                                start=(dc == 0), stop=(dc == NC - 1))
                        nc.scalar.copy(out=V_sb[:, kc, 8 * dvs:8 * dvs + 8, 0:64],
                                       in_=vp[:])
        kvs_cm.__exit__(None, None, None)
        wkp_cm.__exit__(None, None, None)

        # ctx output lives past attention into phase C
        ctxsb_pool = es.enter_context(tc.tile_pool(name="ctxsb", bufs=1))
        ctx_sb = ctxsb_pool.tile([P, NC, QL], BF, tag="ctx_sb")

        # prefetch pools for phase C/D (DMAs issued before attention so they
        # stream during it)
        cres = es.enter_context(tc.tile_pool(name="cres", bufs=1))
        x1 = cres.tile([P, NC, QL], R, tag="x1")   # holds h1, then LN1 output
        wbc = cres.tile([P, NE, QL], BF, tag="wbc")
        wfin = cres.tile([NE, QL], BF, tag="wfin")
        ebt_sb = cres.tile([NE, E], BF, tag="ebt")
        nc.sync.dma_start(ebt_sb[:], ebt[:])
        sels_sb = cres.tile([NE, NE, P], BF, tag="sels")
        nc.sync.dma_start(sels_sb[:], sels[:])
        wo_sb = cres.tile([P, NC, E], BF, tag="wo_sb")
        nc.sync.dma_start(wo_sb[:], wo.rearrange("(c p) m -> p c m", p=P))
        xres_sb = cres.tile([P, NC, QL], BF, tag="xres")
        nc.sync.dma_start(xres_sb[:], xres.rearrange("(c p) q -> p c q", p=P))
        ewp = es.enter_context(tc.tile_pool(name="ewp", bufs=2))
        ew_tiles = []
        for e in range(NE):
            t = ewp.tile([P, NC, E], BF, tag="ew_sl", name=f"ew{e}")
            nc.sync.dma_start(t[:], ew[e].rearrange("c p d -> p c d"))
            ew_tiles.append(t)

        # ===== Phase B: causal attention, hh-paired exp, fused denominator =====
        with tc.tile_pool(name="st_p", bufs=3) as st_p, \
             tc.tile_pool(name="rd_p", bufs=2) as rd_p, \
             tc.tile_pool(name="rb_p", bufs=2) as rb_p, \
             tc.tile_pool(name="scps", bufs=2, space="PSUM") as scps, \
             tc.tile_pool(name="ctxps", bufs=2, space="PSUM") as ctxps:
            for hp in range(NC):
                ctxp2 = [ctxps.tile([65, QL], F, tag=f"ctx{hh}", name=f"ctxp_{hp}_{hh}")
                         for hh in range(2)]
                prev = None
                for kc in range(KCN):
                    j0 = max(0, -(-(kc - qc) // 4))
                    q0 = j0 * P
                    scp = scps.tile([P, 2, QL], F, tag="scp")
                    for hh in range(2):
                        nc.tensor.matmul(
                            scp[:, hh, q0:], kT[hh * 64:(hh + 1) * 64, hp, kc * P:(kc + 1) * P],
                            qT[hh * 64:(hh + 1) * 64, hp, q0:], start=True, stop=True)
                    if prev is not None:
                        pkc, pq0, pst = prev
                        for hh in range(2):
                            nc.tensor.matmul(ctxp2[hh][:, pq0:], V_sb[:, pkc, 2 * hp + hh, :],
                                             pst[:, hh, pq0:],
                                             start=(pkc == 0), stop=False)
                    st = st_p.tile([P, 2, QL], BF, tag="st")
                    nc.scalar.activation(out=st[:, :, q0:], in_=scp[:, :, q0:],
                                         func=AF.Exp, scale=EXP_SCALE)
                    if kc >= qc and (kc - qc) % 4 == 0:
                        j = (kc - qc) // 4
                        for hh in range(2):
                            nc.vector.tensor_tensor(
                                out=st[:, hh, j * P:(j + 1) * P],
                                in0=st[:, hh, j * P:(j + 1) * P],
                                in1=tri_sb[:], op=OP.mult)
                    prev = (kc, q0, st)
                pkc, pq0, pst = prev
                for hh in range(2):
                    nc.tensor.matmul(ctxp2[hh][:, pq0:], V_sb[:, pkc, 2 * hp + hh, :],
                                     pst[:, hh, pq0:], start=(pkc == 0), stop=True)
                for hh in range(2):
                    dsb = rd_p.tile([1, QL], F, tag="dsb")
                    nc.vector.tensor_copy(out=dsb[:], in_=ctxp2[hh][64:65, :])
                    rden = rd_p.tile([1, QL], F, tag="rden")
                    nc.vector.reciprocal_approx_fast(out=rden[:], in_=dsb[:])
                    rb = rb_p.tile([64, QL], F, tag="rb")
                    nc.gpsimd.partition_broadcast(rb[:], rden[:])
                    tmpc = rd_p.tile([64, QL], BF, tag="ctmp")
                    nc.vector.tensor_tensor(
                        out=tmpc[:], in0=ctxp2[hh][0:64, :], in1=rb[:], op=OP.mult)
                    nc.vector.tensor_scalar(
                        out=ctx_sb[hh * 64:(hh + 1) * 64, hp, :], in0=tmpc[:],
                        scalar1=bvp_sb[hh * 64:(hh + 1) * 64, hp:hp + 1], scalar2=None,
                        op0=OP.add)

        # ===== LN helper =====
        def layernorm(src, dst, wtile, btile, tmp, ps_row, post_c=None, cols=slice(None)):
            sp_ = ps_row.tile([1, QL], F, tag="lnrow")
            for c in range(NC):
                nc.tensor.matmul(sp_[:, cols], ones[:], src[:, c, cols],
                                 start=(c == 0), stop=(c == NC - 1))
            s2p = ps_row.tile([1, QL], F, tag="lnrow2")
            for c in range(NC):
                sq = tmp.tile([P, QL], R, tag="lnsq", bufs=2)
                nc.scalar.activation(out=sq[:, cols], in_=src[:, c, cols], func=AF.Square)
                nc.tensor.matmul(s2p[:, cols], ones[:], sq[:, cols],
                                 start=(c == 0), stop=(c == NC - 1))
            mean = tmp.tile([1, QL], F, tag="lnmean")
            nc.scalar.mul(out=mean[:, cols], in_=sp_[:, cols], mul=1.0 / E)
            msq = tmp.tile([1, QL], R, tag="lnmsq")
            nc.vector.tensor_tensor(out=msq[:, cols], in0=mean[:, cols], in1=mean[:, cols], op=OP.mult)
            var = tmp.tile([1, QL], R, tag="lnvar")
            nc.vector.scalar_tensor_tensor(out=var[:, cols], in0=s2p[:, cols], scalar=1.0 / E,
                                           in1=msq[:, cols], op0=OP.mult, op1=OP.subtract)
            lnv = tmp.tile([1, QL], F, tag="lnstd")
            nc.scalar.activation(out=lnv[:, cols], in_=var[:, cols], func=AF.Ln, bias=eps1[:])
            rstd = tmp.tile([1, QL], F, tag="lnrstd")
            nc.scalar.activation(out=rstd[:, cols], in_=lnv[:, cols], func=AF.Exp, scale=-0.5)
            mb = tmp.tile([P, QL], F, tag="lnmb")
            nc.gpsimd.partition_broadcast(mb[:, cols], mean[:, cols])
            rbb = tmp.tile([P, QL], F, tag="lnrb")
            nc.gpsimd.partition_broadcast(rbb[:, cols], rstd[:, cols])
            for c in range(NC):
                t = tmp.tile([P, QL], R, tag="lnt", bufs=2)
                nc.vector.tensor_tensor(out=t[:, cols], in0=src[:, c, cols], in1=mb[:, cols], op=OP.subtract)
                nc.vector.tensor_tensor(out=t[:, cols], in0=t[:, cols], in1=rbb[:, cols], op=OP.mult)
                nc.vector.tensor_scalar(out=dst[:, c, cols], in0=t[:, cols],
                                        scalar1=wtile[:, c:c + 1], scalar2=btile[:, c:c + 1],
                                        op0=OP.mult, op1=OP.add)
                if post_c is not None:
                    post_c(c)

        # ===== Phase C: out-proj + LN1 (in place: h1 -> x1) + gating =====
        with tc.tile_pool(name="ct", bufs=1) as ct, \
             tc.tile_pool(name="cps8", bufs=3, space="PSUM") as cps8:
            for oc in range(NC):
                ap = cps8.tile([P, QL], F, tag="ap", name=f"ap{oc}")
                for dc in range(NC):
                    nc.tensor.matmul(ap[:], wo_sb[:, dc, oc * P:(oc + 1) * P],
                                     ctx_sb[:, dc, :], start=(dc == 0), stop=(dc == NC - 1))
                nc.vector.scalar_tensor_tensor(
                    out=x1[:, oc, :], in0=ap[:], scalar=bo_sb[:, oc:oc + 1],
                    in1=xres_sb[:, oc, :], op0=OP.add, op1=OP.add)
        with tc.tile_pool(name="ct2", bufs=1) as ct, \
             tc.tile_pool(name="cps", bufs=2, space="PSUM") as cps, \
             tc.tile_pool(name="crow", bufs=2, space="PSUM") as crow:
            layernorm(x1, x1, ln_sb["ln1w"], ln_sb["ln1b"], ct, crow)

            # gating in [NE, QL] layout
            gp = crow.tile([NE, QL], F, tag="gps")
            for c in range(NC):
                nc.tensor.matmul(gp[:], gw_sb[:, c, :], x1[:, c, :],
                                 start=(c == 0), stop=(c == NC - 1))
            gexp = ct.tile([NE, QL], R, tag="gexp")
            nc.scalar.activation(out=gexp[:], in_=gp[:], func=AF.Exp, bias=gb_sb[:])
            denp = crow.tile([1, QL], F, tag="gps", name="denp")
            nc.tensor.matmul(denp[:], ones8[:], gexp[:], start=True, stop=True)
            den1 = ct.tile([1, QL], F, tag="den1")
            nc.vector.tensor_copy(out=den1[:], in_=denp[:])
            nc.vector.reciprocal_approx_fast(out=den1[:], in_=den1[:])
            rgb = ct.tile([NE, QL], F, tag="rgb")
            nc.gpsimd.partition_broadcast(rgb[:], den1[:])

            m1b = ct.tile([NE, QL], R, tag="gtmp", bufs=3)
            nc.gpsimd.partition_all_reduce(m1b[:], gexp[:], channels=NE,
                                           reduce_op=bass_isa.ReduceOp.max)
            msel = ct.tile([NE, QL], R, tag="msel")
            nc.vector.tensor_tensor(out=msel[:], in0=gexp[:], in1=m1b[:], op=OP.is_equal)
            inv = ct.tile([NE, QL], R, tag="gtmp", bufs=3)
            nc.vector.tensor_scalar(out=inv[:], in0=msel[:], scalar1=-1.0, scalar2=1.0,
                                    op0=OP.mult, op1=OP.add)
            g2 = ct.tile([NE, QL], R, tag="gtmp", bufs=3)
            nc.vector.tensor_tensor(out=g2[:], in0=gexp[:], in1=inv[:], op=OP.mult)
            m2b = ct.tile([NE, QL], R, tag="gtmp", bufs=3)
            nc.gpsimd.partition_all_reduce(m2b[:], g2[:], channels=NE,
                                           reduce_op=bass_isa.ReduceOp.max)
            msel2 = ct.tile([NE, QL], R, tag="gtmp", bufs=3)
            nc.vector.tensor_tensor(out=msel2[:], in0=g2[:], in1=m2b[:], op=OP.is_equal)
            nc.vector.tensor_tensor(out=msel[:], in0=msel[:], in1=msel2[:], op=OP.add)
            wsel = ct.tile([NE, QL], R, tag="wsel")
            nc.vector.tensor_tensor(out=wsel[:], in0=gexp[:], in1=msel[:], op=OP.mult)
            nc.vector.tensor_tensor(out=wfin[:], in0=wsel[:], in1=rgb[:], op=OP.mult)
            for e in range(NE):
                wbp = cps.tile([P, QL], F, tag="cbig")
                nc.tensor.matmul(wbp[:], sels_sb[:, e, :], wfin[:], start=True, stop=True)
                nc.vector.tensor_copy(out=wbc[:, e, :], in_=wbp[:])

        # ===== Phase D: MoE, experts accumulate in PSUM =====
        h2 = x1  # x1 is dead after the residual add; reuse its SBUF
        with tc.tile_pool(name="x1e_p", bufs=2) as x1e_p, \
             tc.tile_pool(name="dps", bufs=1, space="PSUM") as dps:
            yps = [dps.tile([P, QL], F, tag=f"yp{oc}", name=f"yp{oc}") for oc in range(NC)]
            for oc in range(NC):
                nc.tensor.matmul(yps[oc][:], ebt_sb[:, oc * P:(oc + 1) * P], wfin[:],
                                 start=True, stop=False)
            for e in range(NE):
                ew_sl = ew_tiles[e]
                x1e = [x1e_p.tile([P, QL], BF, tag=f"x1e{dc}", bufs=2, name=f"x1e_{e}_{dc}") for dc in range(NC)]
                for dc in range(NC):
                    nc.vector.tensor_tensor(out=x1e[dc][:], in0=x1[:, dc, :],
                                            in1=wbc[:, e, :], op=OP.mult)
                if e < NE - 1:
                    for dc in range(NC):
                        for oc in range(NC):
                            nc.tensor.matmul(
                                yps[oc][:], ew_sl[:, dc, oc * P:(oc + 1) * P], x1e[dc][:],
                                start=False, stop=False)
                else:
                    # last expert: finish column-half 0 first so h2+LN2 on it
                    # overlap with the half-1 matmuls
                    for h0, h1 in ((0, 256), (256, QL)):
                        for dc in range(NC):
                            for oc in range(NC):
                                nc.tensor.matmul(
                                    yps[oc][:, h0:h1], ew_sl[:, dc, oc * P:(oc + 1) * P],
                                    x1e[dc][:, h0:h1],
                                    start=False, stop=(dc == NC - 1))
            for h0, h1 in ((0, 256), (256, QL)):
                for oc in range(NC):
                    nc.vector.tensor_tensor(out=h2[:, oc, h0:h1], in0=yps[oc][:, h0:h1],
                                            in1=x1[:, oc, h0:h1], op=OP.add)

        # ===== Phase E: LN2 + store =====
        with tc.tile_pool(name="et", bufs=1) as et, \
             tc.tile_pool(name="erow", bufs=2, space="PSUM") as erow:
            out_r = out.rearrange("(c p) q -> p c q", p=P)
            for h0, h1 in ((0, 256), (256, QL)):
                layernorm(h2, h2, ln_sb["ln2w"], ln_sb["ln2b"], et, erow,
                          post_c=(lambda h0=h0, h1=h1: (lambda c: nc.sync.dma_start(
                              out_r[:, c, h0:h1], h2[:, c, h0:h1])))(),
                          cols=slice(h0, h1))

    nc.compile()
    return nc


def _prep_inputs(inputs):
    x = np.asarray(inputs['x'], dtype=np.float32)
    ipw = np.asarray(inputs['in_proj_w'], dtype=np.float32)
    ipb = np.asarray(inputs['in_proj_b'], dtype=np.float32)
    opw = np.asarray(inputs['out_proj_w'], dtype=np.float32)
    opb = np.asarray(inputs['out_proj_b'], dtype=np.float32)
    gww = np.asarray(inputs['gate_w'], dtype=np.float32)
    gbb = np.asarray(inputs['gate_b'], dtype=np.float32)
    eww = np.asarray(inputs['expert_w'], dtype=np.float32)
    ebb = np.asarray(inputs['expert_b'], dtype=np.float32)

    perm = np.empty(E, dtype=np.int64)
    idx = 0
    for h in range(H):
        for i in range(D // 2):
            perm[idx] = 64 * h + 2 * i; idx += 1
    for h in range(H):
        for i in range(D // 2):
            perm[idx] = 64 * h + 2 * i + 1; idx += 1

    Wq, Wk, Wv = ipw[0:E], ipw[E:2 * E], ipw[2 * E:3 * E]
    bq_, bk_, bv_ = ipb[0:E], ipb[E:2 * E], ipb[2 * E:3 * E]
    BFD = ml_dtypes.bfloat16
    common = {
        "wq": np.ascontiguousarray(Wq[:, perm].T).astype(BFD),
        "wk": np.ascontiguousarray(Wk[:, perm].T).astype(BFD),
        "wv": np.ascontiguousarray(Wv[:, perm].T).astype(BFD),
        "bq": np.ascontiguousarray(bq_.reshape(NC, P).T),
        "wo": np.ascontiguousarray(opw.T).astype(BFD),
        "gw": np.ascontiguousarray(gww.T),
        "gb": gbb.reshape(NE, 1).copy(),
        "ew": np.ascontiguousarray(
            eww.transpose(0, 2, 1).reshape(NE, NC, P, E)).astype(BFD),
        "ebt": np.ascontiguousarray(ebb).astype(BFD),
        "trid": np.ascontiguousarray(
            (np.arange(P)[None, :] >= np.arange(P)[:, None])).astype(BFD),
        "sels": np.ascontiguousarray(
            np.repeat(np.eye(NE, dtype=np.float32)[:, :, None], P, axis=2)).astype(BFD),
        "cpack": np.ascontiguousarray(np.concatenate([
            opb.reshape(NC, P).T,
            np.asarray(inputs['ln1_w'], np.float32).reshape(NC, P).T,
            np.asarray(inputs['ln1_b'], np.float32).reshape(NC, P).T,
            np.asarray(inputs['ln2_w'], np.float32).reshape(NC, P).T,
            np.asarray(inputs['ln2_b'], np.float32).reshape(NC, P).T,
            bv_.reshape(NC, P).T,
        ], axis=1)),
    }
    inv_freq = 1.0 / (10000.0 ** (np.arange(0, D, 2, dtype=np.float64) / D))
    freqs = np.arange(S, dtype=np.float64)[:, None] * inv_freq[None, :]
    cos_t = np.cos(freqs).T.astype(np.float32)
    sin_t = np.sin(freqs).T.astype(np.float32)
    cos2 = np.ascontiguousarray(np.tile(cos_t, (4, 1)))
    sin2 = np.ascontiguousarray(np.tile(sin_t, (4, 1)))
    cs2 = np.stack([cos2, sin2], axis=1)  # [P, 2, S]
    common["cs2"] = np.ascontiguousarray(cs2).astype(BFD)

    in_maps = []
    for c in range(8):
        b, qc = c // 4, c % 4
        blocks = [qc + 4 * i for i in range(4)]
        cols = np.concatenate([np.arange(blk * P, (blk + 1) * P) for blk in blocks])
        xtb = np.ascontiguousarray(x[b].T)
        xtp = np.ascontiguousarray(xtb[perm])
        m = dict(common)
        m["xt"] = xtp.astype(BFD)
        m["xtq"] = np.ascontiguousarray(xtp[:, cols]).astype(BFD)
        m["xres"] = np.ascontiguousarray(xtb[:, cols]).astype(BFD)
        m["cs2q"] = np.ascontiguousarray(cs2[:, :, cols]).astype(BFD)
        in_maps.append(m)
    return in_maps


def _run_multi(ncs, in_maps):
    """Run the 4 NEFFs concurrently: graph qc on devices {qc, qc+4} (b=0,1)."""
    import jax
    from jax.sharding import Mesh, PartitionSpec
    from jax.experimental.shard_map import shard_map
    from concourse import bass2jax
    from concourse import mybir as _mb

    bass2jax.install_neuronx_cc_hook()
    devices = jax.devices()

    if "jits" not in _cache:
        _cache["jits"] = {}
    handles = []
    for qc in range(4):
        nc = ncs[qc]
        if qc not in _cache["jits"]:
            in_names, out_names, out_avals, zero_outs = [], [], [], []
            for alloc in nc.m.functions[0].allocations:
                if not isinstance(alloc, _mb.MemoryLocationSet):
                    continue
                name = alloc.memorylocations[0].name
                if alloc.kind == "ExternalInput":
                    in_names.append(name)
                elif alloc.kind == "ExternalOutput":
                    out_names.append(name)
                    shape = tuple(alloc.tensor_shape)
                    dtype = _mb.dt.np(alloc.dtype)
                    out_avals.append(jax.core.ShapedArray(shape, dtype))
                    zero_outs.append(np.zeros(shape, dtype))
            n_params = len(in_names)
            all_names = in_names + out_names
            donate = tuple(range(n_params, n_params + len(out_names)))

            def _body(*args, _nc=nc, _avals=tuple(out_avals), _all=tuple(all_names),
                      _outs=tuple(out_names)):
                outs = bass2jax._bass_exec_p.bind(
                    *args, out_avals=_avals, in_names=_all, out_names=_outs,
                    lowering_input_output_aliases=(),
                    sim_require_finite=True, sim_require_nnan=True, nc=_nc)
                return tuple(outs)

            devs = [devices[qc], devices[qc + 4]]
            mesh = Mesh(np.asarray(devs), ("core",))
            nio = n_params + len(zero_outs)
            sharded = jax.jit(
                shard_map(_body, mesh=mesh,
                          in_specs=(PartitionSpec("core"),) * nio,
                          out_specs=(PartitionSpec("core"),) * len(out_names),
                          check_rep=False),
                donate_argnums=donate, keep_unused=True)
            _cache["jits"][qc] = (sharded, in_names, out_names, zero_outs)
        sharded, in_names, out_names, zero_outs = _cache["jits"][qc]
        per_core = [[np.asarray(in_maps[b * 4 + qc][n]) for n in in_names] for b in range(2)]
        concat_in = [np.concatenate([per_core[b][i] for b in range(2)], axis=0)
                     for i in range(len(in_names))]
        concat_zero = [np.concatenate([z, z], axis=0) for z in zero_outs]
        handles.append((sharded, concat_in, concat_zero, out_names))

    outs = []
    for sharded, concat_in, concat_zero, out_names in handles:
        outs.append((sharded(*concat_in, *concat_zero), out_names))
    results = [None] * 8
    for qc, (arrs, out_names) in enumerate(outs):
        arrs = [np.asarray(a) for a in arrs]
        for b in range(2):
            rm = {}
            for i, n in enumerate(out_names):
                full = arrs[i]
                half = full.shape[0] // 2
                rm[n] = full[b * half:(b + 1) * half]
            results[b * 4 + qc] = rm
    return results


def _ensure_ntff_hook():
    import types
    try:
        from antenv.axon_hooks import get_axon_ntff_profile_hook  # noqa
        return True
    except ImportError:
        pass
    try:
        import antenv
        sys.path.insert(0, '/root/.axon_site')
        from trn_agent_boot.trn_boot import _ntff_profile_via_ctypes
        hook = _ntff_profile_via_ctypes('/opt/axon/libaxon_pjrt.so')
        if hook is None:
            return False
        mod = types.ModuleType('antenv.axon_hooks')
        _state = {'hook': hook}
        mod.set_axon_ntff_profile_hook = lambda h: _state.__setitem__('hook', h)
        mod.get_axon_ntff_profile_hook = lambda: _state['hook']
        sys.modules['antenv.axon_hooks'] = mod
        antenv.axon_hooks = mod
        return True
    except Exception as e:
        print(f"ntff hook setup failed: {e}")
        return False


def kernel(**inputs):
    if "ncs" not in _cache:
        _cache["ncs"] = [_build(qc) for qc in range(4)]
    ncs = _cache["ncs"]
    in_maps = _prep_inputs(inputs)

    trace = bool(int(os.environ.get("KERNEL_TRACE", "0")))
    if trace and _ensure_ntff_hook():
        import tempfile
        from antenv.axon_hooks import get_axon_ntff_profile_hook
        hook = get_axon_ntff_profile_hook()
        tmpdir = tempfile.mkdtemp()
        _run_multi(ncs, in_maps)  # warm-up/compile outside the profile window
        with hook(tmpdir, list(range(8))):
            results = _run_multi(ncs, in_maps)
        _cache["ntff_dir"] = tmpdir
        print(f"ntff dir: {tmpdir}")
    else:
        results = _run_multi(ncs, in_maps)
    _cache["last_results"] = results

    out = np.empty((B, S, E), dtype=np.float32)
    for c in range(8):
        b, qc = c // 4, c % 4
        o = results[c]["out"]  # [E, QL]
        for i in range(4):
            blk = qc + 4 * i
            out[b, blk * P:(blk + 1) * P, :] = o[:, i * P:(i + 1) * P].T
    return out
